# revision 1
# baseline (speedup 1.0000x reference)
"""Multi-head self-attention (B=2, S=2048, D=1024, H=16) on 8 trn2 cores.

Sharding: data-parallel over B (2) x tensor-parallel over head groups (4 groups
of 4 heads).  Core c handles batch c//4, heads (c%4)*4..(c%4)*4+3.  Each core
computes its partial output projection (over its 256 of the 1024 contraction
columns); the host sums the 4 partials per batch and adds the bias terms.

Structure (per core):
  - K/V projections upfront (2-bank PSUM slots, fused evictions).
  - Per 512-query chunk: scores (K=64 matmuls into a [128,2heads,512]
    pair-block), one 1024-wide exp on ScalarE per block, ctx accumulation
    (exp @ [V | ones] gives softmax denominators for free), normalization
    via reciprocal + GpSimd partition-broadcast.
  - The previous chunk's output projection is interleaved into the current
    chunk's kt loop so the in-order PE queue always has independent work
    (keeps the HAM clock gate at 8/8).
"""

import sys

if "/opt/trn_rl_repo" not in sys.path:
    sys.path.insert(0, "/opt/trn_rl_repo")

from contextlib import ExitStack

import ml_dtypes
import numpy as np

import concourse.bass as bass
import concourse.mybir as mybir
import concourse.tile as tile
from concourse import bacc

F32 = mybir.dt.float32
F32R = mybir.dt.float32r
BF16 = mybir.dt.bfloat16
EXP = mybir.ActivationFunctionType.Exp

N_CORES = 8
S = 2048          # sequence length
D = 1024          # model dim
GH = 4            # heads per core
DK = 64           # head dim
E = GH * DK       # projection cols per core (256)
DT = D // 128     # contraction tiles over model dim (8)
KT = S // 128     # key chunks (16)
QC = S // 512     # query chunks (4)


def _build():
    nc = bacc.Bacc("TRN2", target_bir_lowering=False, debug=False,
                   enable_asserts=False, num_devices=N_CORES)

    xT_d = nc.dram_tensor("xT", [D, S], BF16, kind="ExternalInput").ap()
    wqT_d = nc.dram_tensor("wqT", [D, E], BF16, kind="ExternalInput").ap()
    wkT_d = nc.dram_tensor("wkT", [D, E], BF16, kind="ExternalInput").ap()
    wvT_d = nc.dram_tensor("wvT", [D, E], BF16, kind="ExternalInput").ap()
    woT_d = nc.dram_tensor("woT", [E, D], BF16, kind="ExternalInput").ap()
    bq_d = nc.dram_tensor("bq", [E], F32, kind="ExternalInput").ap()
    bk_d = nc.dram_tensor("bk", [E], F32, kind="ExternalInput").ap()
    out_d = nc.dram_tensor("out", [S, D], F32, kind="ExternalOutput").ap()

    with tile.TileContext(nc) as tc, ExitStack() as ctx:
        const = ctx.enter_context(tc.tile_pool(name="const", bufs=1))

        xT_s = const.tile([128, DT, S], BF16, name="xT_s")
        wqT_s = const.tile([128, DT, E], BF16, name="wqT_s")
        wkT_s = const.tile([128, DT, E], BF16, name="wkT_s")
        wvT_s = const.tile([128, DT, E], BF16, name="wvT_s")
        woT_s = const.tile([128, 2, D], BF16, name="woT_s")
        bq_s = const.tile([128, 2], F32, name="bq_s")
        bk_s = const.tile([128, 2], F32, name="bk_s")
        # Q and ctx live in per-qc tiles to avoid cross-qc WAR serialization
        QT_q = [const.tile([128, 2, 512], F32R, name=f"QT{i}") for i in range(QC)]
        ctx_q = [[const.tile([128, 512], BF16, name=f"CT{i}_{p}") for p in range(2)]
                 for i in range(QC)]
        KT_p = [const.tile([128, 1024], F32R, name=f"KTp{i}") for i in range(4)]
        # V' layout: [k-partition, k-chunk, head-major (64 V cols + ones col)]
        Vp_t = [const.tile([128, 4, GH * 128], BF16, name=f"Vp{i}")
                for i in range(4)]

        # DMA order matches first use: (wkT, xT) per d-tile, wvT, wqT, woT.
        # x split in 512KB halves for queue parallelism.
        # Upfront compute (K00, V0/V1, Q0) touches only x cols 0..1024 plus
        # wk/wv/wq -- deliver those first; x quarters 2,3 arrive during the
        # qc0 attention stream (consumed by deferred fillers only).
        def _xq(dt_, quarter):
            eng = nc.sync if quarter % 2 == 0 else nc.gpsimd
            eng.dma_start(
                xT_s[:, dt_, quarter * 512:(quarter + 1) * 512],
                xT_d[dt_ * 128:(dt_ + 1) * 128,
                     quarter * 512:(quarter + 1) * 512])

        nc.sync.dma_start(bk_s[:, :], bk_d.rearrange("(c p) -> p c", p=128))
        for dt_ in range(DT):
            nc.sync.dma_start(wkT_s[:, dt_, :], wkT_d[dt_ * 128:(dt_ + 1) * 128, :])
            _xq(dt_, 0)
            _xq(dt_, 1)
        nc.sync.dma_start(bq_s[:, :], bq_d.rearrange("(c p) -> p c", p=128))
        for dt_ in range(DT):
            nc.sync.dma_start(wvT_s[:, dt_, :], wvT_d[dt_ * 128:(dt_ + 1) * 128, :])
            nc.gpsimd.dma_start(wqT_s[:, dt_, :], wqT_d[dt_ * 128:(dt_ + 1) * 128, :])
        for dt_ in range(DT):
            _xq(dt_, 2)
            _xq(dt_, 3)
        for dc in range(2):
            nc.gpsimd.dma_start(woT_s[:, dc, :], woT_d[dc * 128:(dc + 1) * 128, :])

        # V' per head: [ones, 63 x zero, V(64)] -> denom at psum partition 0,
        # ctx at partitions 64..127 (32-aligned for DVE reads)
        for vt in Vp_t:
            nc.gpsimd.memset(vt[:], 0.0)
            for hh in range(GH):
                nc.vector.memset(vt[:, :, hh * 128], 1.0)

        def kproj_kq(ec, kq, pool, pname):
            ps = pool.tile([128, 2, 512], F32, name=pname)
            for half in range(2):
                qc = kq * 2 + half
                for dt_ in range(DT):
                    nc.tensor.matmul(
                        ps[:, half, :],
                        wkT_s[:, dt_, ec * 128:(ec + 1) * 128],
                        xT_s[:, dt_, qc * 512:(qc + 1) * 512],
                        start=(dt_ == 0), stop=(dt_ == DT - 1))
            nc.vector.tensor_scalar_add(
                KT_p[ec * 2 + kq].rearrange("p (a b) -> p a b", a=2),
                ps[:], bk_s[:, ec:ec + 1])

        def vproj_sg(sg, pool, pname):
            ps = pool.tile([128, 2, 512], F32, name=pname)
            for half in range(2):
                sc = sg * 2 + half
                for dt_ in range(DT):
                    nc.tensor.matmul(
                        ps[:, half, :E],
                        xT_s[:, dt_, sc * 128:(sc + 1) * 128],
                        wvT_s[:, dt_, :],
                        start=(dt_ == 0), stop=(dt_ == DT - 1))
            vt, so = Vp_t[sg // 2], (sg % 2) * 2
            nc.vector.tensor_copy(
                vt[:, so:so + 2, :]
                .rearrange("p s (h d) -> p s h d", d=128)[:, :, :, DK:128],
                ps[:, :, :E].rearrange("p s (h d) -> p s h d", d=DK))

        def qproj_ec(qc, ec, pool=None, pname="sblk"):
            ps = (pool or sp).tile([128, 2, 512], F32, name=pname)
            for dt_ in range(DT):
                nc.tensor.matmul(
                    ps[:, ec, :],
                    wqT_s[:, dt_, ec * 128:(ec + 1) * 128],
                    xT_s[:, dt_, qc * 512:(qc + 1) * 512],
                    start=(dt_ == 0), stop=(dt_ == DT - 1))
            nc.vector.tensor_scalar_add(
                QT_q[qc][:, ec, :], ps[:, ec, :], bq_s[:, ec:ec + 1])

        def qproj(qc, pool=None, pname="sblk"):
            qproj_ec(qc, 0, pool, pname)
            qproj_ec(qc, 1, pool, pname)

        # ---- K / V projections (own 2-slot psum pool, closed afterwards) ----
        with tc.tile_pool(name="kv", bufs=2, space="PSUM") as kv:
            kproj_kq(0, 0, kv, "kvt")
            for sg in range(2):
                vproj_sg(sg, kv, "kvt")
            qproj(0, kv, "kvt")

        # ---- attention + out-projection pipeline over qc ----
        sp = ctx.enter_context(tc.tile_pool(name="sp", bufs=3, space="PSUM"))
        cp = ctx.enter_context(tc.tile_pool(name="cp", bufs=1, space="PSUM"))
        ep = ctx.enter_context(tc.tile_pool(name="ep", bufs=12))
        npool = ctx.enter_context(tc.tile_pool(name="npool", bufs=3))
        op = ctx.enter_context(tc.tile_pool(name="op", bufs=4))

        def outproj_si(qc, si, pool=None, pname="sblk"):
            ssl = slice(si * 128, (si + 1) * 128)
            os_ = op.tile([128, D], F32, name="os_")
            ps = (pool or sp).tile([128, 2, 512], F32, name=pname)
            for eh in range(2):
                for dc in range(2):
                    nc.tensor.matmul(
                        ps[:, eh, :],
                        ctx_q[qc][dc][:, ssl],
                        woT_s[:, dc, eh * 512:(eh + 1) * 512],
                        start=(dc == 0), stop=(dc == 1))
            nc.vector.tensor_copy(os_[:], ps[:].rearrange("p a b -> p (a b)"))
            row = qc * 512 + si * 128
            nc.sync.dma_start(out_d[row:row + 64, :], os_[0:64, :])
            nc.sync.dma_start(out_d[row + 64:row + 128, :], os_[64:128, :])

        DELAY = 6

        def attention(qc, filler, cadence=4, start_u=1):
            """Flat (pair, kt) stream; ctx matmuls delayed DELAY units so the
            in-order PE queue never stalls on exp at pair boundaries.
            filler: callables injected every 4 units."""
            cps = {}
            exs = {}

            def do_scores(u):
                pair, kt = divmod(u, KT)
                if kt == 0:
                    cps[pair] = cp.tile([128, 2, 512], F32, name="cps")
                sblk = sp.tile([128, 2, 512], F32, name="sblk")
                for hi in range(2):
                    po = hi * 64
                    nc.tensor.matmul(
                        sblk[:, hi, :],
                        KT_p[pair * 2 + kt // 8][po:po + 64,
                                                 (kt % 8) * 128:(kt % 8 + 1) * 128],
                        QT_q[qc][po:po + 64, pair, :],
                        start=True, stop=True)
                ex = ep.tile([128, 2, 512], BF16, name="ex")
                nc.scalar.activation(ex[:], sblk[:], EXP, scale=0.125)
                exs[u] = ex

            def do_ctx(u):
                pair, kt = divmod(u, KT)
                ex = exs.pop(u)
                for hi in range(2):
                    h = pair * 2 + hi
                    nc.tensor.matmul(
                        cps[pair][:, hi, :],
                        Vp_t[kt // 4][:, kt % 4, h * 128:(h + 1) * 128],
                        ex[:, hi, :],
                        start=(kt == 0), stop=(kt == KT - 1))
                if kt == KT - 1:
                    finish_pair(pair)

            def finish_pair(pair):
                last = (qc == QC - 1 and pair == 1)
                cc = cps.pop(pair)
                if last:
                    cu = cc  # read psum directly; no next pair needs the slot
                else:
                    cu = npool.tile([128, 2, 512], F32, name="cu")
                    nc.vector.tensor_copy(cu[:], cc[:, :, :])
                rc = npool.tile([1, 1024], F32, name="rc")
                nc.vector.reciprocal_approx_fast(
                    rc[:], cu[0:1, :, :].rearrange("p a b -> p (a b)"))
                bc = npool.tile([128, 1024], F32, name="bc")
                for hi in range(2):
                    nc.gpsimd.partition_broadcast(
                        bc[:, hi * 512:(hi + 1) * 512],
                        rc[:, hi * 512:(hi + 1) * 512])
                    po = hi * 64
                    nc.vector.tensor_mul(
                        ctx_q[qc][pair][po:po + 64, :], cu[64:128, hi, :],
                        bc[64:128, hi * 512:(hi + 1) * 512])

            NU = 2 * KT
            for u in range(NU + DELAY):
                if u < NU:
                    do_scores(u)
                if u >= DELAY:
                    do_ctx(u - DELAY)
                if u % cadence == cadence - 1 and u >= start_u and filler:
                    filler.pop(0)()

        for qc in range(QC):
            filler = []
            cadence = 4
            if qc == 0:
                cadence = 2
                filler = [lambda: kproj_kq(0, 1, sp, "sblk"),
                          lambda: kproj_kq(1, 0, sp, "sblk")]
                filler += [lambda s=sg: vproj_sg(s, sp, "sblk") for sg in range(2, 8)]
                filler.append(lambda: kproj_kq(1, 1, sp, "sblk"))
            else:
                filler = [lambda si=si, q=qc - 1: outproj_si(q, si) for si in range(4)]
            if qc + 1 < QC:
                filler.append(lambda q=qc + 1: qproj_ec(q, 0))
                filler.append(lambda q=qc + 1: qproj_ec(q, 1))
            attention(qc, filler, cadence, start_u=1 if qc == 0 else 7)
        # final chunk: reuse the (idle) scores psum ring to avoid
        # single-slot WAW serialization in the tail
        for si in range(4):
            outproj_si(QC - 1, si, pool=sp, pname="sblk")

    nc.compile()
    return nc


_STATE = {}


def _get_nc():
    if "nc" not in _STATE:
        _STATE["nc"] = _build()
    return _STATE["nc"]


def kernel(x, wq, bq, wk, bk, wv, bv, wo, bo):
    x = np.asarray(x, dtype=np.float32)
    wq = np.asarray(wq, dtype=np.float32)
    bq = np.asarray(bq, dtype=np.float32)
    wk = np.asarray(wk, dtype=np.float32)
    bk = np.asarray(bk, dtype=np.float32)
    wv = np.asarray(wv, dtype=np.float32)
    bv = np.asarray(bv, dtype=np.float32)
    wo = np.asarray(wo, dtype=np.float32)
    bo = np.asarray(bo, dtype=np.float32)

    nc = _get_nc()

    in_maps = []
    for c in range(N_CORES):
        b, g = divmod(c, 4)
        cols = slice(g * E, (g + 1) * E)
        in_maps.append({
            "xT": np.ascontiguousarray(x[b].T).astype(ml_dtypes.bfloat16),
            "wqT": np.ascontiguousarray(wq[cols, :].T).astype(ml_dtypes.bfloat16),
            "wkT": np.ascontiguousarray(wk[cols, :].T).astype(ml_dtypes.bfloat16),
            "wvT": np.ascontiguousarray(wv[cols, :].T).astype(ml_dtypes.bfloat16),
            "woT": np.ascontiguousarray(wo[:, cols].T).astype(ml_dtypes.bfloat16),
            "bq": np.ascontiguousarray(bq[cols]),
            "bk": np.ascontiguousarray(bk[cols]),
        })

    from concourse import bass_utils
    res = bass_utils.run_bass_kernel_spmd(
        nc, in_maps, core_ids=list(range(N_CORES)), trace=False)

    bias = (bo + wo @ bv).astype(np.float32)
    out = np.empty((2, S, D), dtype=np.float32)
    for b in range(2):
        acc = res.results[b * 4 + 0]["out"].astype(np.float64)
        for g in range(1, 4):
            acc += res.results[b * 4 + g]["out"]
        out[b] = (acc + bias).astype(np.float32)
    return out

